# Initial kernel scaffold
#
"""DynamicScatter (voxel mean, reduce='mean') on 8 Trainium2 NeuronCores.

Contract: kernel(points[1e6,64] f32, coors[1e6,3] i32) ->
          (voxel_feats [1e6,64] f32, voxel_coors [1e6,3] i32)
matching reference: unique linearized voxel ids (ascending), per-voxel mean
features, padded to N rows with feats=0 / coors=-1.

Sharding: points are routed to cores by coors[:,0] // 8 (spatial partition of
the 64^3 voxel grid into 8 slabs); each core owns a dense 32768-voxel slab
accumulator. Device does the scatter-reduce (sums + counts); host does the
final mean + compaction + padding.
"""

import numpy as np

from concourse import bass, mybir
import concourse.tile as tile
from concourse.vector_clock import ScopedClock
from concourse.masks import make_identity, make_upper_triangular
from concourse.bass_utils import run_bass_kernel_spmd

F32 = mybir.dt.float32
I32 = mybir.dt.int32

N_POINTS = 1_000_000
FEATS = 64
GRID = 64
N_CORES = 8
C0_PER_CORE = GRID // N_CORES          # 8 c0-planes per core
V_CORE = C0_PER_CORE * GRID * GRID     # 32768 voxels per core
SHARD = 131072                         # padded points per core (1024 tiles)
N_CHAINS = 4                           # round-robin accumulator tables
P = 128
FC = FEATS + 1
DROP = 1 << 20                         # idx offset that fails bounds check


def _patch_tile_drain():
    """This container's walrus accepts only 1 sync-wait per CTRL-class
    instruction; split the kernel-tail drain's waits across single-wait
    NOPs."""
    if getattr(tile.TileContext, "_drain_patched", False):
        return

    def _drain_and_barrier(self, tick_clock, wait_clock):
        nc = self.nc
        drain_inst = nc.sync.drain()
        wait_clock.add_sem_waits(
            drain_inst.ins, ScopedClock({None: tick_clock.global_clock})
        )
        si = drain_inst.ins.sync_info
        waits = list(si.on_wait or [])
        if len(waits) > 1:
            si.on_wait = [waits[0]]
            for w in waits[1:]:
                nop = nc.sync.nop(nofuse=True)
                nop.ins.sync_info = mybir.SyncInfo(on_wait=[w], on_update=[])
        nc.all_engine_barrier()
        popped = nc._tile_sem_poison_stack.pop()
        assert popped is self._sem_poison
        nc.clear_and_free_semaphores(list(self.sems.allocated().values()))
        nc.all_engine_barrier()

    tile.TileContext._drain_and_barrier = _drain_and_barrier
    tile.TileContext._drain_patched = True


def _split_multiwait_instructions(nc):
    """Hoist excess sem waits (>1 per instruction) onto same-engine NOPs."""
    from bass_rust import InstNoOp

    for fn in nc.m.functions:
        for bb in fn.blocks:
            new_insts = []
            for ins in bb.instructions:
                si = getattr(ins, "sync_info", None)
                waits = list(si.on_wait) if si and si.on_wait else []
                if len(waits) > 1:
                    for w in waits[:-1]:
                        nop = InstNoOp(
                            name=nc.get_next_instruction_name(),
                            engine=ins.engine,
                            ins=[],
                            outs=[],
                            sync_info=mybir.SyncInfo(on_wait=[w], on_update=[]),
                        )
                        new_insts.append(nop)
                    si.on_wait = [waits[-1]]
                new_insts.append(ins)
            bb.instructions[:] = new_insts


def build_kernel():
    """Per-core program. Inputs: pts [SHARD, 65] f32 (feats+ones, pads zero),
    keyi [SHARD, 1] i32 (local voxel id, pads DROP). Outputs: acc{k}
    [V_CORE, 65] f32 sums+count."""
    _patch_tile_drain()
    n_tiles = SHARD // P

    nc = bass.Bass("TRN2", target_bir_lowering=False)
    pts = nc.dram_tensor("pts", [SHARD, FC], F32, kind="ExternalInput")
    keyi = nc.dram_tensor("keyi", [SHARD, 1], I32, kind="ExternalInput")
    accs = [
        nc.dram_tensor(f"acc{k}", [V_CORE, FC], F32, kind="ExternalOutput")
        for k in range(N_CHAINS)
    ]

    with tile.TileContext(nc) as tc:
        with (
            tc.tile_pool(name="const", bufs=1) as constp,
            tc.tile_pool(name="sb", bufs=4) as sb,
            tc.tile_pool(name="ps", bufs=2, space="PSUM") as ps,
        ):
            ident = constp.tile([P, P], F32)
            make_identity(nc, ident[:])
            ustrict = constp.tile([P, P], F32)
            make_upper_triangular(nc, ustrict[:], val=1.0, diag=False)
            ones_col = constp.tile([P, 1], F32)
            nc.gpsimd.memset(ones_col[:], 1.0)

            for t in range(n_tiles):
                k = t % N_CHAINS
                pts_t = sb.tile([P, FC], F32, tag="pts_t")
                nc.sync.dma_start(out=pts_t[:], in_=pts[t * P:(t + 1) * P, :])
                ki_t = sb.tile([P, 1], I32, tag="ki_t")
                nc.sync.dma_start(out=ki_t[:], in_=keyi[t * P:(t + 1) * P, :])

                kf = sb.tile([P, 1], F32, tag="kf")
                nc.vector.tensor_copy(out=kf[:], in_=ki_t[:])

                kT_ps = ps.tile([P, P], F32, tag="kT_ps", space="PSUM")
                nc.tensor.transpose(
                    out=kT_ps[:], in_=kf[:].to_broadcast([P, P]), identity=ident[:]
                )
                kT = sb.tile([P, P], F32, tag="kT")
                nc.scalar.copy(out=kT[:], in_=kT_ps[:])
                meq = sb.tile([P, P], F32, tag="meq")
                nc.vector.tensor_tensor(
                    out=meq[:],
                    in0=kf[:].to_broadcast([P, P]),
                    in1=kT[:],
                    op=mybir.AluOpType.is_equal,
                )
                mequ = sb.tile([P, P], F32, tag="mequ")
                nc.vector.tensor_tensor(
                    out=mequ[:], in0=meq[:], in1=ustrict[:], op=mybir.AluOpType.mult
                )
                r_ps = ps.tile([P, 1], F32, tag="r_ps", space="PSUM")
                nc.tensor.matmul(
                    out=r_ps[:], lhsT=mequ[:], rhs=ones_col[:], start=True, stop=True
                )
                s_ps = ps.tile([P, FC], F32, tag="s_ps", space="PSUM")
                nc.tensor.matmul(
                    out=s_ps[:], lhsT=meq[:], rhs=pts_t[:], start=True, stop=True
                )
                s_sb = sb.tile([P, FC], F32, tag="s_sb")
                nc.scalar.copy(out=s_sb[:], in_=s_ps[:])

                leader = sb.tile([P, 1], F32, tag="leader")
                nc.vector.tensor_scalar(
                    out=leader[:],
                    in0=r_ps[:],
                    scalar1=0.0,
                    scalar2=None,
                    op0=mybir.AluOpType.is_equal,
                )
                idx_f = sb.tile([P, 1], F32, tag="idx_f")
                nc.vector.tensor_scalar(
                    out=idx_f[:],
                    in0=leader[:],
                    scalar1=float(-DROP),
                    scalar2=float(DROP),
                    op0=mybir.AluOpType.mult,
                    op1=mybir.AluOpType.add,
                )
                nc.vector.tensor_tensor(
                    out=idx_f[:], in0=idx_f[:], in1=kf[:], op=mybir.AluOpType.add
                )
                idx_i = sb.tile([P, 1], I32, tag="idx_i")
                nc.vector.tensor_copy(out=idx_i[:], in_=idx_f[:])

                nc.gpsimd.indirect_dma_start(
                    out=accs[k][:],
                    out_offset=bass.IndirectOffsetOnAxis(ap=idx_i[:, :1], axis=0),
                    in_=s_sb[:],
                    in_offset=None,
                    bounds_check=V_CORE - 1,
                    oob_is_err=False,
                    compute_op=mybir.AluOpType.add,
                )

    _split_multiwait_instructions(nc)
    return nc


_NC_CACHE = None


def _get_nc():
    global _NC_CACHE
    if _NC_CACHE is None:
        _NC_CACHE = build_kernel()
    return _NC_CACHE


def _shard_inputs(points, coors):
    """Route points to cores by c0//8; pad each shard to SHARD rows."""
    points = np.asarray(points, dtype=np.float32)
    coors = np.asarray(coors, dtype=np.int32)
    lin = (
        coors[:, 0].astype(np.int64) * (GRID * GRID)
        + coors[:, 1].astype(np.int64) * GRID
        + coors[:, 2].astype(np.int64)
    )
    core = coors[:, 0] >> 3
    order = np.argsort(core, kind="stable")
    core_sorted = core[order]
    bounds = np.searchsorted(core_sorted, np.arange(N_CORES + 1))
    in_maps = []
    for i in range(N_CORES):
        sel = order[bounds[i]:bounds[i + 1]]
        n = sel.shape[0]
        assert n <= SHARD, f"core {i} got {n} > {SHARD} points"
        pts = np.zeros((SHARD, FC), dtype=np.float32)
        pts[:n, :FEATS] = points[sel]
        pts[:n, FEATS] = 1.0
        keyi = np.full((SHARD, 1), DROP, dtype=np.int32)
        keyi[:n, 0] = (lin[sel] - i * V_CORE).astype(np.int32)
        in_maps.append({"pts": pts, "keyi": keyi})
    return in_maps


def _assemble(results):
    """Sum chain tables, compute means, compact ascending-voxel-id rows."""
    acc = np.zeros((N_CORES * V_CORE, FC), dtype=np.float32)
    for i in range(N_CORES):
        a = results[i]["acc0"].copy()
        for k in range(1, N_CHAINS):
            a += results[i][f"acc{k}"]
        acc[i * V_CORE:(i + 1) * V_CORE] = a
    counts = acc[:, FEATS]
    valid = counts > 0
    nv = int(valid.sum())
    vids = np.nonzero(valid)[0]
    voxel_feats = np.zeros((N_POINTS, FEATS), dtype=np.float32)
    voxel_feats[:nv] = acc[valid, :FEATS] / counts[valid, None]
    voxel_coors = np.full((N_POINTS, 3), -1, dtype=np.int32)
    voxel_coors[:nv, 0] = vids // (GRID * GRID)
    voxel_coors[:nv, 1] = (vids // GRID) % GRID
    voxel_coors[:nv, 2] = vids % GRID
    return voxel_feats, voxel_coors


def kernel(points, coors, _return_results=False):
    nc = _get_nc()
    in_maps = _shard_inputs(points, coors)
    res = run_bass_kernel_spmd(nc, in_maps, core_ids=list(range(N_CORES)))
    out = _assemble(res.results)
    if _return_results:
        return out, res
    return out


# revision 16
# speedup vs baseline: 2.1315x; 2.1315x over previous
"""DynamicScatter (voxel mean, reduce='mean') on 8 Trainium2 NeuronCores.

Contract: kernel(points[1e6,64] f32, coors[1e6,3] i32) ->
          (voxel_feats [1e6,64] f32, voxel_coors [1e6,3] i32)
matching reference: unique linearized voxel ids (ascending), per-voxel mean
features, padded to N rows with feats=0 / coors=-1.

Sharding: points are routed to cores by coors[:,0] // 8 (spatial partition of
the 64^3 voxel grid into 8 slabs); each core owns a dense 32768-voxel slab.
Device computes per-voxel sums+counts via per-tile duplicate-combining
(selection-matrix matmul) and batched indirect scatter-add DMAs; host does
the final mean + compaction + padding.

Batching layout: G tiles share one scatter DMA. Tile g of a batch targets
row range [g*(V+1), g*(V+1)+V] of its chain table, so descriptors within one
DMA can never collide on a row (in-DMA RMW races lose updates). Non-leader
rows carry zero payload and go to each slice's dump row V.
"""

import numpy as np

from concourse import bass, mybir
import concourse.tile as tile
from concourse.vector_clock import ScopedClock
from concourse.masks import make_identity, make_upper_triangular
from concourse.bass_utils import run_bass_kernel_spmd

F32 = mybir.dt.float32
I32 = mybir.dt.int32

N_POINTS = 1_000_000
FEATS = 64
GRID = 64
N_CORES = 8
C0_PER_CORE = GRID // N_CORES          # 8 c0-planes per core
V_CORE = C0_PER_CORE * GRID * GRID     # 32768 voxels per core
SHARD = 131072                         # padded points per core (1024 tiles)
N_LAUNCHES = 1
G = 4                                  # tiles batched per scatter DMA
N_CHAINS = 2                           # round-robin table pairs
P = 128
RC = FEATS + 2                         # row: feats, one, key(f32)
SC = FEATS + 1                         # scattered row: feats, count
R_SLICE = V_CORE + 1                   # rows per batch-position slice


def _patch_tile_drain():
    """This container's walrus accepts only 1 sync-wait per CTRL-class
    instruction; split the kernel-tail drain's waits across single-wait
    NOPs."""
    if getattr(tile.TileContext, "_drain_patched", False):
        return

    def _drain_and_barrier(self, tick_clock, wait_clock):
        nc = self.nc
        drain_inst = nc.sync.drain()
        wait_clock.add_sem_waits(
            drain_inst.ins, ScopedClock({None: tick_clock.global_clock})
        )
        si = drain_inst.ins.sync_info
        waits = list(si.on_wait or [])
        if len(waits) > 1:
            si.on_wait = [waits[0]]
            for w in waits[1:]:
                nop = nc.sync.nop(nofuse=True)
                nop.ins.sync_info = mybir.SyncInfo(on_wait=[w], on_update=[])
        nc.all_engine_barrier()
        popped = nc._tile_sem_poison_stack.pop()
        assert popped is self._sem_poison
        nc.clear_and_free_semaphores(list(self.sems.allocated().values()))
        nc.all_engine_barrier()

    tile.TileContext._drain_and_barrier = _drain_and_barrier
    tile.TileContext._drain_patched = True


def _split_multiwait_instructions(nc):
    """Hoist excess sem waits (>1 per instruction) onto same-engine NOPs."""
    from bass_rust import InstNoOp

    for fn in nc.m.functions:
        for bb in fn.blocks:
            new_insts = []
            for ins in bb.instructions:
                si = getattr(ins, "sync_info", None)
                waits = list(si.on_wait) if si and si.on_wait else []
                if len(waits) > 1:
                    for w in waits[:-1]:
                        nop = InstNoOp(
                            name=nc.get_next_instruction_name(),
                            engine=ins.engine,
                            ins=[],
                            outs=[],
                            sync_info=mybir.SyncInfo(on_wait=[w], on_update=[]),
                        )
                        new_insts.append(nop)
                    si.on_wait = [waits[-1]]
                new_insts.append(ins)
            bb.instructions[:] = new_insts


def build_kernel():
    """Per-core program.
    Input:  pts [SHARD, RC] f32 rows = [64 feats, 1.0, key]; pad rows are
            all-zero (key 0, weight 0).
    Output: acc{k} [G*R_SLICE, SC] f32 — batch-position slices of sums+count.
    """
    _patch_tile_drain()
    n_tiles = SHARD // P
    n_batches = n_tiles // G
    assert SHARD % (P * G) == 0

    nc = bass.Bass("TRN2", target_bir_lowering=False)
    pts = nc.dram_tensor("pts", [SHARD, RC], F32, kind="ExternalInput")
    accs = [
        nc.dram_tensor(f"acc{k}", [G * R_SLICE, SC], F32, kind="ExternalOutput")
        for k in range(N_CHAINS)
    ]

    with tile.TileContext(nc) as tc:
        with (
            tc.tile_pool(name="const", bufs=1) as constp,
            tc.tile_pool(name="sb", bufs=4) as sb,
            tc.tile_pool(name="ps", bufs=2, space="PSUM") as ps,
        ):
            ident = constp.tile([P, P], F32)
            make_identity(nc, ident[:])
            ustrict = constp.tile([P, P], F32)
            make_upper_triangular(nc, ustrict[:], val=1.0, diag=False)
            ones_col = constp.tile([P, 1], F32)
            nc.gpsimd.memset(ones_col[:], 1.0)

            for b in range(n_batches):
                k = b % N_CHAINS
                # one DMA loads G tiles: point (g*128+p, c) -> [p, g*RC + c]
                pts_b = sb.tile([P, G * RC], F32, tag="pts_b")
                src = pts[b * G * P:(b + 1) * G * P, :].rearrange(
                    "(g p) c -> p g c", g=G, p=P
                )
                dst = pts_b[:].rearrange("p (g c) -> p g c", g=G, c=RC)
                nc.sync.dma_start(out=dst, in_=src)

                s_b = sb.tile([P, G * SC], F32, tag="s_b")
                idx_b = sb.tile([P, G], I32, tag="idx_b")

                for g in range(G):
                    col = g * RC
                    kf = pts_b[:, col + FEATS + 1:col + RC]  # [P,1] key f32

                    kT_ps = ps.tile([P, P], F32, tag="kT_ps", space="PSUM")
                    nc.tensor.transpose(
                        out=kT_ps[:], in_=kf.to_broadcast([P, P]), identity=ident[:]
                    )
                    kT = sb.tile([P, P], F32, tag="kT")
                    nc.scalar.copy(out=kT[:], in_=kT_ps[:])
                    meq = sb.tile([P, P], F32, tag="meq")
                    nc.vector.tensor_tensor(
                        out=meq[:],
                        in0=kf.to_broadcast([P, P]),
                        in1=kT[:],
                        op=mybir.AluOpType.is_equal,
                    )
                    mequ = sb.tile([P, P], F32, tag="mequ")
                    nc.vector.tensor_tensor(
                        out=mequ[:], in0=meq[:], in1=ustrict[:],
                        op=mybir.AluOpType.mult,
                    )
                    # rank r_m = #(p<m with same key); leader = (r == 0)
                    r_ps = ps.tile([P, 1], F32, tag="r_ps", space="PSUM")
                    nc.tensor.matmul(
                        out=r_ps[:], lhsT=mequ[:], rhs=ones_col[:],
                        start=True, stop=True,
                    )
                    # group sums (feats + count) for every member row
                    s_ps = ps.tile([P, SC], F32, tag="s_ps", space="PSUM")
                    nc.tensor.matmul(
                        out=s_ps[:], lhsT=meq[:],
                        rhs=pts_b[:, col:col + SC],
                        start=True, stop=True,
                    )
                    # leader = relu(1 - r)  (r is a small non-negative int)
                    leader = sb.tile([P, 1], F32, tag="leader")
                    nc.scalar.activation(
                        out=leader[:], in_=r_ps[:],
                        func=mybir.ActivationFunctionType.Relu,
                        bias=1.0, scale=-1.0,
                    )
                    # zero non-leader rows while copying PSUM->SBUF
                    # (per-partition scale = leader)
                    nc.scalar.activation(
                        out=s_b[:, g * SC:(g + 1) * SC], in_=s_ps[:],
                        func=mybir.ActivationFunctionType.Copy,
                        scale=leader[:],
                    )
                    # idx = leader*(key - V) + (V + g*R_SLICE): leaders hit
                    # their slice row, non-leaders the slice dump row
                    t_f = sb.tile([P, 1], F32, tag="t_f")
                    nc.vector.tensor_scalar(
                        out=t_f[:], in0=kf, scalar1=float(-V_CORE),
                        scalar2=None, op0=mybir.AluOpType.add,
                    )
                    nc.vector.tensor_tensor(
                        out=t_f[:], in0=t_f[:], in1=leader[:],
                        op=mybir.AluOpType.mult,
                    )
                    nc.vector.tensor_scalar(
                        out=idx_b[:, g:g + 1], in0=t_f[:],
                        scalar1=float(V_CORE + g * R_SLICE),
                        scalar2=None, op0=mybir.AluOpType.add,
                    )

                nc.gpsimd.indirect_dma_start(
                    out=accs[k][:],
                    out_offset=bass.IndirectOffsetOnAxis(ap=idx_b[:, :G], axis=0),
                    in_=s_b[:],
                    in_offset=None,
                    compute_op=mybir.AluOpType.add,
                )

    _split_multiwait_instructions(nc)
    return nc


_NC_CACHE = None


def _get_nc():
    global _NC_CACHE
    if _NC_CACHE is None:
        _NC_CACHE = build_kernel()
    return _NC_CACHE


def _shard_inputs(points, coors):
    """Route points to cores by c0//8, split across launches, pad to SHARD.
    Returns list of N_LAUNCHES lists of per-core input dicts."""
    points = np.asarray(points, dtype=np.float32)
    coors = np.asarray(coors, dtype=np.int32)
    lin = (
        coors[:, 0].astype(np.int64) * (GRID * GRID)
        + coors[:, 1].astype(np.int64) * GRID
        + coors[:, 2].astype(np.int64)
    )
    core = coors[:, 0] >> 3
    order = np.argsort(core, kind="stable")
    core_sorted = core[order]
    bounds = np.searchsorted(core_sorted, np.arange(N_CORES + 1))
    launches = [[] for _ in range(N_LAUNCHES)]
    for i in range(N_CORES):
        sel = order[bounds[i]:bounds[i + 1]]
        for li, piece in enumerate(np.array_split(sel, N_LAUNCHES)):
            n = piece.shape[0]
            assert n <= SHARD, f"core {i} launch {li} got {n} > {SHARD} points"
            pts = np.zeros((SHARD, RC), dtype=np.float32)
            pts[:n, :FEATS] = points[piece]
            pts[:n, FEATS] = 1.0
            pts[:n, FEATS + 1] = (lin[piece] - i * V_CORE).astype(np.float32)
            launches[li].append({"pts": pts})
    return launches


def _assemble(all_results):
    """Sum chain tables and batch-position slices over launches, compute
    means, compact ascending-voxel-id rows."""
    acc = np.zeros((N_CORES * V_CORE, SC), dtype=np.float32)
    for i in range(N_CORES):
        a = np.zeros((V_CORE, SC), dtype=np.float32)
        for results in all_results:
            for k in range(N_CHAINS):
                t = results[i][f"acc{k}"].reshape(G, R_SLICE, SC)
                a += t[:, :V_CORE, :].sum(axis=0)
        acc[i * V_CORE:(i + 1) * V_CORE] = a
    counts = acc[:, FEATS]
    valid = counts > 0
    nv = int(valid.sum())
    vids = np.nonzero(valid)[0]
    voxel_feats = np.zeros((N_POINTS, FEATS), dtype=np.float32)
    voxel_feats[:nv] = acc[valid, :FEATS] / counts[valid, None]
    voxel_coors = np.full((N_POINTS, 3), -1, dtype=np.int32)
    voxel_coors[:nv, 0] = vids // (GRID * GRID)
    voxel_coors[:nv, 1] = (vids // GRID) % GRID
    voxel_coors[:nv, 2] = vids % GRID
    return voxel_feats, voxel_coors


def kernel(points, coors, _return_results=False):
    nc = _get_nc()
    launches = _shard_inputs(points, coors)
    all_res = [
        run_bass_kernel_spmd(nc, in_maps, core_ids=list(range(N_CORES)))
        for in_maps in launches
    ]
    out = _assemble([r.results for r in all_res])
    if _return_results:
        return out, all_res
    return out
